# revision 15
# baseline (speedup 1.0000x reference)
"""Distributed Trainium2 kernel for gnn_message_passing (nn_AMN_18004502905276).

Reference computation:
    masked = where(conn > 0.1, conn, 0)          # [64, 64]
    w      = 3.0 * masked.sum(axis=0)            # [64]
    out    = einsum('j,jtn->tn', w, unit_outputs)  # [100, 4096]

Strategy: shard along N (4096 = 8 x 512) so every core computes its own
output slice with zero collectives.  Per core the weighted unit-sum is a
[128,2]^T @ [128,400] bf16 matmul: the moving operand stacks two 64-unit
groups on the 128 partitions, the stationary operand is a block-diagonal
copy of w, so each matmul consumes 800 data columns.  w is computed on
device (mask + ones-matmul into PSUM via two PE column-groups).  Inputs
are pre-rounded to bf16 host-side (the matmul consumes bf16 either way),
halving the HBM stream.  The stream is 8 HWDGE DMAs of [128, 3200] bf16
(6.4 KB/partition packets); each group feeds 8 matmuls cycling the 8 PSUM
banks; VectorE copies banks 0-3, ScalarE banks 4-7 (one strided 1600-col
copy each); GpSimd drains each group's [2, 3200] output slice to DRAM as
bf16.  Four PE column-groups (partition pairs 0/32/64/96) spread the
output rows across two SBUF ports.
"""

import contextlib
import sys

import numpy as np

sys.path.insert(0, "/opt/trn_rl_repo")

import concourse.bass as bass
import concourse.mybir as mybir
from concourse.bass_utils import run_bass_kernel_spmd

# Problem geometry (hardcoded per the harness contract).
U, T, N = 64, 100, 4096
NCORES = 8
NS = N // NCORES          # 512 output columns per core
FLAT = T * NS             # 51200 flat (t, n) positions per core
GROUP_F = 3200            # moving columns per DMA group half
NGROUPS = FLAT // (2 * GROUP_F)  # 8
MM_F = 400                # moving columns per matmul
MMS_PER_GROUP = 2 * GROUP_F // (2 * MM_F)  # 8, cycling all 8 PSUM banks
NB = 4                    # group buffers in the SBUF ring
F32 = mybir.dt.float32
BF16 = mybir.dt.bfloat16

THRESHOLD = 0.1
STRENGTH = 3.0


def build_nc() -> bass.Bass:
    nc = bass.Bass()

    x_d = nc.declare_dram_parameter("x", [NGROUPS, 128, GROUP_F], BF16, isOutput=False)
    conn_d = nc.declare_dram_parameter("conn", [U, U], F32, isOutput=False)
    out_d = nc.declare_dram_parameter("out", [8, 2 * GROUP_F], BF16, isOutput=True)

    ctx = contextlib.ExitStack()
    with ctx:
        xb = ctx.enter_context(nc.sbuf_tensor("xb", [128, NB * GROUP_F], BF16))
        conn_sb = ctx.enter_context(nc.sbuf_tensor([U, U], F32))
        masked = ctx.enter_context(nc.sbuf_tensor([U, U], F32))
        ones_sb = ctx.enter_context(nc.sbuf_tensor([U, 1], F32))
        s_sb = ctx.enter_context(nc.sbuf_tensor([128, 2], BF16))
        out_sb = ctx.enter_context(nc.sbuf_tensor([128, 2 * GROUP_F], BF16))
        psum = ctx.enter_context(nc.psum_tensor([128, 4096], F32))

        ctx.enter_context(nc.Block())
        block = nc.cur_block
        dma_c = ctx.enter_context(nc.semaphore("dma_c"))
        dma_x = [
            ctx.enter_context(nc.semaphore(f"dma_x{i}")) for i in range(NGROUPS)
        ]
        dma_out = ctx.enter_context(nc.semaphore("dma_out"))
        mm_sem = ctx.enter_context(nc.semaphore("mm_sem"))
        ve_sem = ctx.enter_context(nc.semaphore("ve_sem"))
        s_sem = ctx.enter_context(nc.semaphore("s_sem"))
        cpv_sem = ctx.enter_context(nc.semaphore("cpv_sem"))
        cps_sem = ctx.enter_context(nc.semaphore("cps_sem"))

        def pair_rows(j):
            q = j // 2
            return 32 * q, 32 * q + 2

        def copy_aps(j, half):
            """(psum_ap, out_ap) for banks [4*half, 4*half+4) of group j."""
            r0, r1 = pair_rows(j)
            pb = psum[r0:r1, half * 2048 : (half + 1) * 2048]
            src = pb.rearrange("p (b r) -> p b r", r=512)[:, :, 0:MM_F]
            c0 = (j % 2) * GROUP_F + half * 4 * MM_F
            dst = out_sb[r0:r1, c0 : c0 + 4 * MM_F].rearrange(
                "p (b r) -> p b r", r=MM_F
            )
            return src, dst

        @block.sync
        def _(sync):
            sync.dma_start(out=conn_sb[:, :], in_=conn_d[:, :]).then_inc(dma_c, 16)
            for j in range(NGROUPS):
                if j >= NB:
                    # all 8 matmuls of group j-NB done -> ring slot is free
                    sync.wait_ge(mm_sem, 2 + 8 * (j - NB) + 8)
                s0 = (j % NB) * GROUP_F
                sync.dma_start(
                    out=xb[:, s0 : s0 + GROUP_F], in_=x_d[j]
                ).then_inc(dma_x[j], 16)

        @block.vector
        def _(vector):
            vector.memset(ones_sb[:, :], 1.0).then_inc(ve_sem)
            vector.memset(s_sb[:, :], 0.0).then_inc(ve_sem)
            vector.wait_ge(dma_c, 16)
            # masked = (conn > 0.1) * conn
            vector.scalar_tensor_tensor(
                out=masked[:, :],
                in0=conn_sb[:, :],
                scalar=THRESHOLD,
                in1=conn_sb[:, :],
                op0=mybir.AluOpType.is_gt,
                op1=mybir.AluOpType.mult,
            ).then_inc(ve_sem)
            # S[0:64, 0] = 3 * w ; S[64:128, 1] = 3 * w  (block diagonal)
            vector.wait_ge(mm_sem, 2)
            vector.tensor_scalar_mul(s_sb[0:64, 0:1], psum[0:64, 0:1], STRENGTH
                                     ).then_inc(s_sem)
            vector.tensor_scalar_mul(s_sb[64:128, 1:2], psum[64:128, 0:1], STRENGTH
                                     ).then_inc(s_sem)
            for j in range(NGROUPS):
                vector.wait_ge(mm_sem, 2 + 8 * j + 4)
                src, dst = copy_aps(j, 0)
                vector.tensor_copy(out=dst, in_=src).then_inc(cpv_sem)

        @block.scalar
        def _(scalar):
            for j in range(NGROUPS):
                scalar.wait_ge(mm_sem, 2 + 8 * j + 8)
                src, dst = copy_aps(j, 1)
                scalar.copy(dst, src).then_inc(cps_sem)

        @block.tensor
        def _(tensor):
            tensor.wait_ge(ve_sem, 3)
            # w[j] = sum_i masked[i, j], materialized on partitions 0-63 and 64-127
            tensor.matmul(
                psum[0:64, 0:1], masked[:, :], ones_sb[:, :], start=True, stop=True
            ).then_inc(mm_sem)
            tensor.matmul(
                psum[64:128, 0:1],
                masked[:, :],
                ones_sb[:, :],
                start=True,
                stop=True,
                tile_position=(0, 64),
            ).then_inc(mm_sem)
            tensor.wait_ge(s_sem, 2)
            for j in range(NGROUPS):
                tensor.wait_ge(dma_x[j], 16)
                r0, r1 = pair_rows(j)
                s0 = (j % NB) * GROUP_F
                for m in range(MMS_PER_GROUP):
                    if m == 0 and j >= 1:
                        tensor.wait_ge(cpv_sem, j)  # banks 0-3 of j-1 copied
                    if m == 4 and j >= 1:
                        tensor.wait_ge(cps_sem, j)  # banks 4-7 of j-1 copied
                    tensor.matmul(
                        psum[r0:r1, m * 512 : m * 512 + MM_F],
                        s_sb[:, :],
                        xb[:, s0 + m * MM_F : s0 + (m + 1) * MM_F],
                        start=True,
                        stop=True,
                        tile_position=(0, 32 * (j // 2)),
                    ).then_inc(mm_sem)

        @block.gpsimd
        def _(gpsimd):
            # per-group output drain: group j's 8 matmul results are the
            # contiguous cols [(j%2)*3200, +3200) of pair q = j//2
            for j in range(NGROUPS):
                gpsimd.wait_ge(cpv_sem, j + 1)
                gpsimd.wait_ge(cps_sem, j + 1)
                r0, r1 = pair_rows(j)
                c0 = (j % 2) * GROUP_F
                gpsimd.dma_start(
                    out=out_d[2 * (j // 2) : 2 * (j // 2) + 2, c0 : c0 + GROUP_F],
                    in_=out_sb[r0:r1, c0 : c0 + GROUP_F],
                ).then_inc(dma_out, 16)
            gpsimd.wait_ge(dma_out, 16 * NGROUPS)

    return nc


def shard_inputs(unit_outputs: np.ndarray, conn: np.ndarray):
    """Full inputs -> per-core in_maps with the group layout the kernel expects.

    Shards are pre-rounded to bf16 (what the device matmul consumes anyway)
    so the HBM stream moves half the bytes.
    """
    import ml_dtypes

    conn = np.ascontiguousarray(conn, dtype=np.float32)
    in_maps = []
    for c in range(NCORES):
        xc = np.ascontiguousarray(
            unit_outputs[:, :, c * NS : (c + 1) * NS], dtype=np.float32
        ).reshape(U, FLAT)
        # [u, j, h, f] -> [j, (h u), f]
        v = xc.reshape(U, NGROUPS, 2, GROUP_F)
        tiles = np.ascontiguousarray(
            v.transpose(1, 2, 0, 3).astype(ml_dtypes.bfloat16)
        ).reshape(NGROUPS, 128, GROUP_F)
        in_maps.append({"x": tiles, "conn": conn})
    return in_maps


def unshard_output(results) -> np.ndarray:
    """Per-core [8, 6400] bf16 outputs -> full [T, N] f32."""
    final = np.empty((T, N), dtype=np.float32)
    for c in range(NCORES):
        r = np.asarray(results[c]["out"]).astype(np.float32)
        # row 2q+h, col dj*3200 + m*400 + cc  for group j = 2q+dj
        arr = r.reshape(4, 2, 2, 8, MM_F)  # [q, h, dj, m, cc]
        flat = arr.transpose(0, 2, 1, 3, 4).reshape(FLAT)  # [j, h, m, cc] order
        final[:, c * NS : (c + 1) * NS] = flat.reshape(T, NS)
    return final


_NC_CACHE = None


def kernel(unit_outputs: np.ndarray, conn: np.ndarray) -> np.ndarray:
    global _NC_CACHE
    if _NC_CACHE is None:
        _NC_CACHE = build_nc()
    in_maps = shard_inputs(unit_outputs, conn)
    res = run_bass_kernel_spmd(_NC_CACHE, in_maps, core_ids=list(range(NCORES)))
    return unshard_output(res.results)


if __name__ == "__main__":
    rng = np.random.default_rng(0)
    uo = rng.random((U, T, N), dtype=np.float32)
    cn = rng.random((U, U), dtype=np.float32)
    out = kernel(uo, cn)
    w = np.where(cn > THRESHOLD, cn, 0.0).sum(axis=0) * STRENGTH
    ref = np.einsum("j,jtn->tn", w, uo)
    err = np.abs(out - ref).max() / np.abs(ref).max()
    print("rel err:", err)


# revision 18
# speedup vs baseline: 1.1391x; 1.1391x over previous
"""Distributed Trainium2 kernel for gnn_message_passing (nn_AMN_18004502905276).

Reference computation:
    masked = where(conn > 0.1, conn, 0)          # [64, 64]
    w      = 3.0 * masked.sum(axis=0)            # [64]
    out    = einsum('j,jtn->tn', w, unit_outputs)  # [100, 4096]

Strategy: shard along N (4096 = 8 x 512) so every core computes its own
output slice with zero collectives.  Per core the weighted unit-sum is a
[128,2]^T @ [128,400] bf16 matmul: the moving operand stacks two 64-unit
groups on the 128 partitions, the stationary operand is a block-diagonal
copy of w (computed on device from conn).  Inputs are pre-rounded to bf16
host-side (the matmul consumes bf16 either way), halving the HBM stream.

Per core: 8 HWDGE DMAs of [128, 3200] bf16 feed 8 matmuls each.  Matmul m
rotates over PE column groups 32*(m%4), so four consecutive matmuls write
the SAME PSUM bank at partition pairs 0/32/64/96 — one [98, 400] DVE copy
then drains all four results at once (lanes in between copy junk that is
never DMAed).  VectorE copies even banks, ScalarE odd banks; GpSimd
streams each group's output rows to DRAM as bf16.  A burst of dummy
matmuls at kernel start warms the PE (HAM K=8/8) before real work.
"""

import contextlib
import sys

import numpy as np

sys.path.insert(0, "/opt/trn_rl_repo")

import concourse.bass as bass
import concourse.mybir as mybir
from concourse.bass_utils import run_bass_kernel_spmd

# Problem geometry (hardcoded per the harness contract).
U, T, N = 64, 100, 4096
NCORES = 8
NS = N // NCORES          # 512 output columns per core
FLAT = T * NS             # 51200 flat (t, n) positions per core
GROUP_F = 3200            # moving columns per DMA group half
NGROUPS = FLAT // (2 * GROUP_F)  # 8
MM_F = 400                # moving columns per matmul
NB = 4                    # group buffers in the SBUF ring
N_WARMUP = 8              # dummy matmuls to flip HAM to K=8/8
F32 = mybir.dt.float32
BF16 = mybir.dt.bfloat16

THRESHOLD = 0.1
STRENGTH = 3.0


def build_nc() -> bass.Bass:
    nc = bass.Bass()

    x_d = nc.declare_dram_parameter("x", [NGROUPS, 128, GROUP_F], BF16, isOutput=False)
    conn_d = nc.declare_dram_parameter("conn", [U, U], F32, isOutput=False)
    out_d = nc.declare_dram_parameter("out", [8, 2 * GROUP_F], BF16, isOutput=True)

    ctx = contextlib.ExitStack()
    with ctx:
        xb = ctx.enter_context(nc.sbuf_tensor("xb", [128, NB * GROUP_F], BF16))
        dummy = ctx.enter_context(nc.sbuf_tensor([128, 512], BF16))
        conn_sb = ctx.enter_context(nc.sbuf_tensor([U, U], F32))
        masked = ctx.enter_context(nc.sbuf_tensor([U, U], F32))
        ones_sb = ctx.enter_context(nc.sbuf_tensor([U, 1], F32))
        s_sb = ctx.enter_context(nc.sbuf_tensor([128, 2], BF16))
        out_sb = ctx.enter_context(nc.sbuf_tensor([128, 2 * GROUP_F], BF16))
        psum = ctx.enter_context(nc.psum_tensor([128, 4096], F32))

        ctx.enter_context(nc.Block())
        block = nc.cur_block
        dma_c = ctx.enter_context(nc.semaphore("dma_c"))
        dma_x = [
            ctx.enter_context(nc.semaphore(f"dma_x{i}")) for i in range(NGROUPS)
        ]
        dma_out = ctx.enter_context(nc.semaphore("dma_out"))
        mm_sem = ctx.enter_context(nc.semaphore("mm_sem"))
        ve_sem = ctx.enter_context(nc.semaphore("ve_sem"))
        s_sem = ctx.enter_context(nc.semaphore("s_sem"))
        cpv_sem = ctx.enter_context(nc.semaphore("cpv_sem"))
        cps_sem = ctx.enter_context(nc.semaphore("cps_sem"))

        def bank_of(j, half):
            return (2 * j + half) % 8

        def copy_aps(j, half):
            """All four pairs' results for bank (2j+half)%8 of group j."""
            b = bank_of(j, half)
            src = psum[0:98, b * 512 : b * 512 + MM_F]
            c0 = j * 2 * MM_F + half * MM_F
            dst = out_sb[0:98, c0 : c0 + MM_F]
            return src, dst

        @block.scalar
        def _(scalar):
            # conn load on the ACT HWDGE ring so the SP ring starts on x
            scalar.dma_start(out=conn_sb[:, :], in_=conn_d[:, :]).then_inc(dma_c, 16)
            for j in range(NGROUPS):
                scalar.wait_ge(mm_sem, 2 + 8 * j + 8)
                src, dst = copy_aps(j, 1)
                scalar.copy(dst, src).then_inc(cps_sem)

        @block.sync
        def _(sync):
            for j in range(NGROUPS):
                if j >= NB:
                    # all 8 matmuls of group j-NB done -> ring slot is free
                    sync.wait_ge(mm_sem, 2 + 8 * (j - NB) + 8)
                s0 = (j % NB) * GROUP_F
                sync.dma_start(
                    out=xb[:, s0 : s0 + GROUP_F], in_=x_d[j]
                ).then_inc(dma_x[j], 16)

        @block.vector
        def _(vector):
            vector.memset(dummy[:, :], 0.0).then_inc(ve_sem)
            vector.memset(ones_sb[:, :], 1.0).then_inc(ve_sem)
            vector.memset(s_sb[:, :], 0.0).then_inc(ve_sem)
            vector.wait_ge(dma_c, 16)
            # masked = (conn > 0.1) * conn
            vector.scalar_tensor_tensor(
                out=masked[:, :],
                in0=conn_sb[:, :],
                scalar=THRESHOLD,
                in1=conn_sb[:, :],
                op0=mybir.AluOpType.is_gt,
                op1=mybir.AluOpType.mult,
            ).then_inc(ve_sem)
            # S[0:64, 0] = 3 * w ; S[64:128, 1] = 3 * w  (block diagonal)
            vector.wait_ge(mm_sem, 2)
            vector.tensor_scalar_mul(s_sb[0:64, 0:1], psum[0:64, 0:1], STRENGTH
                                     ).then_inc(s_sem)
            vector.tensor_scalar_mul(s_sb[64:128, 1:2], psum[64:128, 0:1], STRENGTH
                                     ).then_inc(s_sem)
            for j in range(NGROUPS):
                vector.wait_ge(mm_sem, 2 + 8 * j + 4)
                src, dst = copy_aps(j, 0)
                vector.tensor_copy(out=dst, in_=src).then_inc(cpv_sem)

        @block.tensor
        def _(tensor):
            # HAM warmup: ~3.5us of dummy matmuls so real work runs at 2.4 GHz.
            # M=98 also zero-fills psum[0:98] of every bank, which the wide
            # drain copies read (rows between the col-group pairs are junk).
            tensor.wait_ge(ve_sem, 1)
            for i in range(N_WARMUP):
                b = i % 8
                tensor.matmul(
                    psum[0:98, b * 512 : (b + 1) * 512],
                    dummy[:, 0:98],
                    dummy[:, :],
                    start=True,
                    stop=True,
                )
            tensor.wait_ge(ve_sem, 4)
            # w[j] = sum_i masked[i, j], materialized on partitions 0-63 and 64-127
            tensor.matmul(
                psum[0:64, 0:1], masked[:, :], ones_sb[:, :], start=True, stop=True
            ).then_inc(mm_sem)
            tensor.matmul(
                psum[64:128, 0:1],
                masked[:, :],
                ones_sb[:, :],
                start=True,
                stop=True,
                tile_position=(0, 64),
            ).then_inc(mm_sem)
            tensor.wait_ge(s_sem, 2)
            for j in range(NGROUPS):
                tensor.wait_ge(dma_x[j], 16)
                s0 = (j % NB) * GROUP_F
                for m in range(8):
                    if j >= 4:
                        # bank (2j+m//4)%8 was last filled by group j-4 and
                        # drained by that group's copy
                        if m == 0:
                            tensor.wait_ge(cpv_sem, j - 3)
                        if m == 4:
                            tensor.wait_ge(cps_sem, j - 3)
                    p = m % 4
                    b = bank_of(j, m // 4)
                    tensor.matmul(
                        psum[32 * p : 32 * p + 2, b * 512 : b * 512 + MM_F],
                        s_sb[:, :],
                        xb[:, s0 + m * MM_F : s0 + (m + 1) * MM_F],
                        start=True,
                        stop=True,
                        tile_position=(0, 32 * p),
                    ).then_inc(mm_sem)

        @block.gpsimd
        def _(gpsimd):
            # stream each group's 8 result rows out as they complete
            for j in range(NGROUPS):
                gpsimd.wait_ge(cpv_sem, j + 1)
                gpsimd.wait_ge(cps_sem, j + 1)
                c0, c1 = j * 2 * MM_F, (j + 1) * 2 * MM_F
                for p in range(4):
                    gpsimd.dma_start(
                        out=out_d[2 * p : 2 * p + 2, c0:c1],
                        in_=out_sb[32 * p : 32 * p + 2, c0:c1],
                    ).then_inc(dma_out, 16)
            gpsimd.wait_ge(dma_out, 64 * NGROUPS)

    return nc


def shard_inputs(unit_outputs: np.ndarray, conn: np.ndarray):
    """Full inputs -> per-core in_maps with the group layout the kernel expects.

    Shards are pre-rounded to bf16 (what the device matmul consumes anyway)
    so the HBM stream moves half the bytes.
    """
    import ml_dtypes

    conn = np.ascontiguousarray(conn, dtype=np.float32)
    in_maps = []
    for c in range(NCORES):
        xc = np.ascontiguousarray(
            unit_outputs[:, :, c * NS : (c + 1) * NS], dtype=np.float32
        ).reshape(U, FLAT)
        # [u, j, h, f] -> [j, (h u), f]
        v = xc.reshape(U, NGROUPS, 2, GROUP_F)
        tiles = np.ascontiguousarray(
            v.transpose(1, 2, 0, 3).astype(ml_dtypes.bfloat16)
        ).reshape(NGROUPS, 128, GROUP_F)
        in_maps.append({"x": tiles, "conn": conn})
    return in_maps


def unshard_output(results) -> np.ndarray:
    """Per-core [8, 6400] bf16 outputs -> full [T, N] f32.

    Row 2p+h holds matmul m in {p, 4+p} of each group j at columns
    j*800 + (m//4)*400, i.e. flat = j*6400 + h*3200 + m*400 + cc.
    """
    final = np.empty((T, N), dtype=np.float32)
    for c in range(NCORES):
        r = np.asarray(results[c]["out"]).astype(np.float32)
        arr = r.reshape(4, 2, NGROUPS, 2, MM_F)  # [p, h, j, b2, cc]
        flat = arr.transpose(2, 1, 3, 0, 4).reshape(FLAT)  # [j, h, b2, p, cc]
        final[:, c * NS : (c + 1) * NS] = flat.reshape(T, NS)
    return final


_NC_CACHE = None


def kernel(unit_outputs: np.ndarray, conn: np.ndarray) -> np.ndarray:
    global _NC_CACHE
    if _NC_CACHE is None:
        _NC_CACHE = build_nc()
    in_maps = shard_inputs(unit_outputs, conn)
    res = run_bass_kernel_spmd(_NC_CACHE, in_maps, core_ids=list(range(NCORES)))
    return unshard_output(res.results)


if __name__ == "__main__":
    rng = np.random.default_rng(0)
    uo = rng.random((U, T, N), dtype=np.float32)
    cn = rng.random((U, U), dtype=np.float32)
    out = kernel(uo, cn)
    w = np.where(cn > THRESHOLD, cn, 0.0).sum(axis=0) * STRENGTH
    ref = np.einsum("j,jtn->tn", w, uo)
    err = np.abs(out - ref).max() / np.abs(ref).max()
    print("rel err:", err)


# revision 24
# speedup vs baseline: 1.2124x; 1.0644x over previous
"""Distributed Trainium2 kernel for gnn_message_passing (nn_AMN_18004502905276).

Reference computation:
    masked = where(conn > 0.1, conn, 0)          # [64, 64]
    w      = 3.0 * masked.sum(axis=0)            # [64]
    out    = einsum('j,jtn->tn', w, unit_outputs)  # [100, 4096]

Strategy: shard along N (4096 = 8 x 512) so every core computes its own
output slice with zero collectives.  Per core the weighted unit-sum is a
[128,2]^T @ [128,400] bf16 matmul: the moving operand stacks two 64-unit
groups on the 128 partitions, the stationary operand is a block-diagonal
copy of w (computed on device from conn).  Inputs are pre-rounded to bf16
host-side (the matmul consumes bf16 either way), halving the HBM stream.

Per core: 8 HWDGE DMAs of [128, 3200] bf16 feed 8 matmuls each.  Matmul m
rotates over PE column groups 32*(m%4), so four consecutive matmuls write
the SAME PSUM bank at partition pairs 0/32/64/96 — one [98, 400] DVE copy
then drains all four results at once (lanes in between copy junk that is
never DMAed).  VectorE copies even banks, ScalarE odd banks; GpSimd
streams each group's output rows to DRAM as bf16.  A burst of dummy
matmuls at kernel start warms the PE (HAM K=8/8) before real work.
"""

import contextlib
import sys

import numpy as np

sys.path.insert(0, "/opt/trn_rl_repo")

import concourse.bass as bass
import concourse.mybir as mybir
from concourse.bass_utils import run_bass_kernel_spmd

# Problem geometry (hardcoded per the harness contract).
U, T, N = 64, 100, 4096
NCORES = 8
NS = N // NCORES          # 512 output columns per core
FLAT = T * NS             # 51200 flat (t, n) positions per core
GROUP_F = 3200            # moving columns per DMA group half
NGROUPS = FLAT // (2 * GROUP_F)  # 8
MM_F = 400                # moving columns per matmul
NB = 4                    # group buffers in the SBUF ring
N_WARMUP = 8              # dummy matmuls to flip HAM to K=8/8; one per
                          # PSUM bank so the wide drain copies read no
                          # uninitialized rows
F32 = mybir.dt.float32
BF16 = mybir.dt.bfloat16

THRESHOLD = 0.1
STRENGTH = 3.0


def build_nc() -> bass.Bass:
    nc = bass.Bass()

    x_d = nc.declare_dram_parameter("x", [NGROUPS, 128, GROUP_F], BF16, isOutput=False)
    conn_d = nc.declare_dram_parameter("conn", [U, U], F32, isOutput=False)
    out_d = nc.declare_dram_parameter("out", [8, 2 * GROUP_F], BF16, isOutput=True)

    ctx = contextlib.ExitStack()
    with ctx:
        xb = ctx.enter_context(nc.sbuf_tensor("xb", [128, NB * GROUP_F], BF16))
        dummy = ctx.enter_context(nc.sbuf_tensor([128, 512], BF16))
        conn_sb = ctx.enter_context(nc.sbuf_tensor([U, U], F32))
        masked = ctx.enter_context(nc.sbuf_tensor([U, U], F32))
        ones_sb = ctx.enter_context(nc.sbuf_tensor([U, 1], F32))
        s_sb = ctx.enter_context(nc.sbuf_tensor([128, 2], BF16))
        out_sb = ctx.enter_context(nc.sbuf_tensor([128, 2 * GROUP_F], BF16))
        psum = ctx.enter_context(nc.psum_tensor([128, 4096], F32))

        ctx.enter_context(nc.Block())
        block = nc.cur_block
        dma_c = ctx.enter_context(nc.semaphore("dma_c"))
        dma_x = [
            ctx.enter_context(nc.semaphore(f"dma_x{i}")) for i in range(NGROUPS)
        ]
        dma_os = ctx.enter_context(nc.semaphore("dma_os"))
        dma_oa = ctx.enter_context(nc.semaphore("dma_oa"))
        mm_sem = ctx.enter_context(nc.semaphore("mm_sem"))
        ve_sem = ctx.enter_context(nc.semaphore("ve_sem"))
        s_sem = ctx.enter_context(nc.semaphore("s_sem"))
        cpv_sem = ctx.enter_context(nc.semaphore("cpv_sem"))
        cps_sem = ctx.enter_context(nc.semaphore("cps_sem"))

        def bank_of(j, half):
            return (2 * j + half) % 8

        def copy_aps(j, half):
            """All four pairs' results for bank (2j+half)%8 of group j."""
            b = bank_of(j, half)
            src = psum[0:98, b * 512 : b * 512 + MM_F]
            c0 = j * 2 * MM_F + half * MM_F
            dst = out_sb[0:98, c0 : c0 + MM_F]
            return src, dst

        @block.scalar
        def _(scalar):
            # conn load on the ACT HWDGE ring so the SP ring starts on x
            scalar.dma_start(out=conn_sb[:, :], in_=conn_d[:, :]).then_inc(dma_c, 16)
            for j in range(NGROUPS):
                scalar.wait_ge(mm_sem, 2 + 8 * j + 8)
                src, dst = copy_aps(j, 1)
                scalar.copy(dst, src).then_inc(cps_sem)
                # drain pairs 2,3 of group j on the ACT HWDGE ring
                # (self-wait: the DMA must not race scalar's in-flight copy)
                scalar.wait_ge(cps_sem, j + 1)
                scalar.wait_ge(cpv_sem, j + 1)
                c0, c1 = j * 2 * MM_F, (j + 1) * 2 * MM_F
                for p in (2, 3):
                    scalar.dma_start(
                        out=out_d[2 * p : 2 * p + 2, c0:c1],
                        in_=out_sb[32 * p : 32 * p + 2, c0:c1],
                    ).then_inc(dma_oa, 16)
            scalar.wait_ge(dma_oa, 32 * NGROUPS)

        @block.sync
        def _(sync):
            for j in range(NGROUPS):
                if j >= NB:
                    # all 8 matmuls of group j-NB done -> ring slot is free
                    sync.wait_ge(mm_sem, 2 + 8 * (j - NB) + 8)
                s0 = (j % NB) * GROUP_F
                sync.dma_start(
                    out=xb[:, s0 : s0 + GROUP_F], in_=x_d[j]
                ).then_inc(dma_x[j], 16)
            # drain pairs 0,1 on the SP HWDGE ring once all loads are queued
            for j in range(NGROUPS):
                sync.wait_ge(cpv_sem, j + 1)
                sync.wait_ge(cps_sem, j + 1)
                c0, c1 = j * 2 * MM_F, (j + 1) * 2 * MM_F
                for p in (0, 1):
                    sync.dma_start(
                        out=out_d[2 * p : 2 * p + 2, c0:c1],
                        in_=out_sb[32 * p : 32 * p + 2, c0:c1],
                    ).then_inc(dma_os, 16)
            sync.wait_ge(dma_os, 32 * NGROUPS)

        @block.vector
        def _(vector):
            vector.memset(dummy[:, :], 0.0).then_inc(ve_sem)
            vector.memset(ones_sb[:, :], 1.0).then_inc(ve_sem)
            vector.memset(s_sb[:, :], 0.0).then_inc(ve_sem)
            vector.wait_ge(dma_c, 16)
            # masked = (conn > 0.1) * conn
            vector.scalar_tensor_tensor(
                out=masked[:, :],
                in0=conn_sb[:, :],
                scalar=THRESHOLD,
                in1=conn_sb[:, :],
                op0=mybir.AluOpType.is_gt,
                op1=mybir.AluOpType.mult,
            ).then_inc(ve_sem)
            # S[0:64, 0] = 3 * w ; S[64:128, 1] = 3 * w  (block diagonal)
            vector.wait_ge(mm_sem, 2)
            vector.tensor_scalar_mul(s_sb[0:64, 0:1], psum[0:64, 0:1], STRENGTH
                                     ).then_inc(s_sem)
            vector.tensor_scalar_mul(s_sb[64:128, 1:2], psum[64:128, 0:1], STRENGTH
                                     ).then_inc(s_sem)
            for j in range(NGROUPS):
                vector.wait_ge(mm_sem, 2 + 8 * j + 4)
                src, dst = copy_aps(j, 0)
                vector.tensor_copy(out=dst, in_=src).then_inc(cpv_sem)

        @block.tensor
        def _(tensor):
            # HAM warmup: ~3.5us of dummy matmuls so real work runs at 2.4 GHz.
            # M=98 also zero-fills psum[0:98] of every bank, which the wide
            # drain copies read (rows between the col-group pairs are junk).
            tensor.wait_ge(ve_sem, 1)
            for i in range(N_WARMUP):
                b = i % 8
                tensor.matmul(
                    psum[0:98, b * 512 : (b + 1) * 512],
                    dummy[:, 0:98],
                    dummy[:, :],
                    start=True,
                    stop=True,
                )
            tensor.wait_ge(ve_sem, 4)
            # w[j] = sum_i masked[i, j], materialized on partitions 0-63 and 64-127
            tensor.matmul(
                psum[0:64, 0:1], masked[:, :], ones_sb[:, :], start=True, stop=True
            ).then_inc(mm_sem)
            tensor.matmul(
                psum[64:128, 0:1],
                masked[:, :],
                ones_sb[:, :],
                start=True,
                stop=True,
                tile_position=(0, 64),
            ).then_inc(mm_sem)
            tensor.wait_ge(s_sem, 2)
            for j in range(NGROUPS):
                tensor.wait_ge(dma_x[j], 16)
                s0 = (j % NB) * GROUP_F
                for m in range(8):
                    if j >= 4:
                        # bank (2j+m//4)%8 was last filled by group j-4 and
                        # drained by that group's copy
                        if m == 0:
                            tensor.wait_ge(cpv_sem, j - 3)
                        if m == 4:
                            tensor.wait_ge(cps_sem, j - 3)
                    p = m % 4
                    b = bank_of(j, m // 4)
                    tensor.matmul(
                        psum[32 * p : 32 * p + 2, b * 512 : b * 512 + MM_F],
                        s_sb[:, :],
                        xb[:, s0 + m * MM_F : s0 + (m + 1) * MM_F],
                        start=True,
                        stop=True,
                        tile_position=(0, 32 * p),
                    ).then_inc(mm_sem)



    return nc


def shard_inputs(unit_outputs: np.ndarray, conn: np.ndarray):
    """Full inputs -> per-core in_maps with the group layout the kernel expects.

    Shards are pre-rounded to bf16 (what the device matmul consumes anyway)
    so the HBM stream moves half the bytes.
    """
    import ml_dtypes

    conn = np.ascontiguousarray(conn, dtype=np.float32)
    in_maps = []
    for c in range(NCORES):
        xc = np.ascontiguousarray(
            unit_outputs[:, :, c * NS : (c + 1) * NS], dtype=np.float32
        ).reshape(U, FLAT)
        # [u, j, h, f] -> [j, (h u), f]
        v = xc.reshape(U, NGROUPS, 2, GROUP_F)
        tiles = np.ascontiguousarray(
            v.transpose(1, 2, 0, 3).astype(ml_dtypes.bfloat16)
        ).reshape(NGROUPS, 128, GROUP_F)
        in_maps.append({"x": tiles, "conn": conn})
    return in_maps


def unshard_output(results) -> np.ndarray:
    """Per-core [8, 6400] bf16 outputs -> full [T, N] f32.

    Row 2p+h holds matmul m in {p, 4+p} of each group j at columns
    j*800 + (m//4)*400, i.e. flat = j*6400 + h*3200 + m*400 + cc.
    """
    final = np.empty((T, N), dtype=np.float32)
    for c in range(NCORES):
        r = np.asarray(results[c]["out"]).astype(np.float32)
        arr = r.reshape(4, 2, NGROUPS, 2, MM_F)  # [p, h, j, b2, cc]
        flat = arr.transpose(2, 1, 3, 0, 4).reshape(FLAT)  # [j, h, b2, p, cc]
        final[:, c * NS : (c + 1) * NS] = flat.reshape(T, NS)
    return final


_NC_CACHE = None


def kernel(unit_outputs: np.ndarray, conn: np.ndarray) -> np.ndarray:
    global _NC_CACHE
    if _NC_CACHE is None:
        _NC_CACHE = build_nc()
    in_maps = shard_inputs(unit_outputs, conn)
    res = run_bass_kernel_spmd(_NC_CACHE, in_maps, core_ids=list(range(NCORES)))
    return unshard_output(res.results)


if __name__ == "__main__":
    rng = np.random.default_rng(0)
    uo = rng.random((U, T, N), dtype=np.float32)
    cn = rng.random((U, U), dtype=np.float32)
    out = kernel(uo, cn)
    w = np.where(cn > THRESHOLD, cn, 0.0).sum(axis=0) * STRENGTH
    ref = np.einsum("j,jtn->tn", w, uo)
    err = np.abs(out - ref).max() / np.abs(ref).max()
    print("rel err:", err)


# revision 25
# speedup vs baseline: 1.3566x; 1.1190x over previous
"""Distributed Trainium2 kernel for gnn_message_passing (nn_AMN_18004502905276).

Reference computation:
    masked = where(conn > 0.1, conn, 0)          # [64, 64]
    w      = 3.0 * masked.sum(axis=0)            # [64]
    out    = einsum('j,jtn->tn', w, unit_outputs)  # [100, 4096]

Strategy: shard along N (4096 = 8 x 512) so every core computes its own
output slice with zero collectives.  Per core the weighted unit-sum is a
[128,2]^T @ [128,400] bf16 matmul: the moving operand stacks two 64-unit
groups on the 128 partitions, the stationary operand is a block-diagonal
copy of w (computed on device from conn).  Inputs are pre-rounded to bf16
host-side (the matmul consumes bf16 either way), halving the HBM stream.

Per core: 4 HWDGE DMAs of [128, 6400] bf16 feed 16 matmuls each.  Matmul
m rotates over PE column groups 32*(m%4) and banks (4j+m//4)%8, so four
consecutive matmuls write the SAME PSUM bank at partition pairs
0/32/64/96 — one [98, 400] DVE copy then drains all four results at once
(lanes in between move junk that is never DMAed).  VectorE copies the
first two banks of each group, ScalarE the other two; four [2, 6400]
HWDGE DMAs at the end write the result rows to DRAM as bf16.  A burst of
dummy matmuls at kernel start warms the PE (HAM K=8/8) before real work;
they also zero psum[0:98] of every bank for the wide copies.
"""

import contextlib
import sys

import numpy as np

sys.path.insert(0, "/opt/trn_rl_repo")

import concourse.bass as bass
import concourse.mybir as mybir
from concourse.bass_utils import run_bass_kernel_spmd

# Problem geometry (hardcoded per the harness contract).
U, T, N = 64, 100, 4096
NCORES = 8
NS = N // NCORES          # 512 output columns per core
FLAT = T * NS             # 51200 flat (t, n) positions per core
GROUP_F = 6400            # moving columns per DMA group half
NGROUPS = FLAT // (2 * GROUP_F)  # 4
MM_F = 400                # moving columns per matmul
MPG = 16                  # matmuls per group
NB = 2                    # group buffers in the SBUF ring
N_WARMUP = 8              # dummy matmuls: HAM warmup + PSUM bank init
F32 = mybir.dt.float32
BF16 = mybir.dt.bfloat16

THRESHOLD = 0.1
STRENGTH = 3.0


def build_nc() -> bass.Bass:
    nc = bass.Bass()

    x_d = nc.declare_dram_parameter("x", [NGROUPS, 128, GROUP_F], BF16, isOutput=False)
    conn_d = nc.declare_dram_parameter("conn", [U, U], F32, isOutput=False)
    out_d = nc.declare_dram_parameter("out", [8, 6400], BF16, isOutput=True)

    ctx = contextlib.ExitStack()
    with ctx:
        xb = ctx.enter_context(nc.sbuf_tensor("xb", [128, NB * GROUP_F], BF16))
        dummy = ctx.enter_context(nc.sbuf_tensor([128, 512], BF16))
        conn_sb = ctx.enter_context(nc.sbuf_tensor([U, U], F32))
        masked = ctx.enter_context(nc.sbuf_tensor([U, U], F32))
        ones_sb = ctx.enter_context(nc.sbuf_tensor([U, 1], F32))
        s_sb = ctx.enter_context(nc.sbuf_tensor([128, 2], BF16))
        out_sb = ctx.enter_context(nc.sbuf_tensor([128, 6400], BF16))
        psum = ctx.enter_context(nc.psum_tensor([128, 4096], F32))

        ctx.enter_context(nc.Block())
        block = nc.cur_block
        dma_c = ctx.enter_context(nc.semaphore("dma_c"))
        dma_x = [
            ctx.enter_context(nc.semaphore(f"dma_x{i}")) for i in range(NGROUPS)
        ]
        dma_os = ctx.enter_context(nc.semaphore("dma_os"))
        dma_oa = ctx.enter_context(nc.semaphore("dma_oa"))
        mm_sem = ctx.enter_context(nc.semaphore("mm_sem"))
        ve_sem = ctx.enter_context(nc.semaphore("ve_sem"))
        s_sem = ctx.enter_context(nc.semaphore("s_sem"))
        cpv_sem = ctx.enter_context(nc.semaphore("cpv_sem"))
        cps_sem = ctx.enter_context(nc.semaphore("cps_sem"))

        def copy_aps(j, b2):
            """All four pairs' results for bank (4j+b2)%8 of group j."""
            b = (4 * j + b2) % 8
            src = psum[0:98, b * 512 : b * 512 + MM_F]
            c0 = j * 4 * MM_F + b2 * MM_F
            dst = out_sb[0:98, c0 : c0 + MM_F]
            return src, dst

        @block.scalar
        def _(scalar):
            # conn load on the ACT HWDGE ring so the SP ring starts on x
            scalar.dma_start(out=conn_sb[:, :], in_=conn_d[:, :]).then_inc(dma_c, 16)
            for j in range(NGROUPS):
                for b2 in (2, 3):
                    scalar.wait_ge(mm_sem, 2 + MPG * j + 4 * (b2 + 1))
                    src, dst = copy_aps(j, b2)
                    scalar.copy(dst, src).then_inc(cps_sem)
            # final drain of pairs 2,3 (self-wait so the DMA doesn't race
            # scalar's own in-flight copies)
            scalar.wait_ge(cps_sem, 2 * NGROUPS)
            scalar.wait_ge(cpv_sem, 2 * NGROUPS)
            for p in (2, 3):
                scalar.dma_start(
                    out=out_d[2 * p : 2 * p + 2, :],
                    in_=out_sb[32 * p : 32 * p + 2, :],
                ).then_inc(dma_oa, 16)
            scalar.wait_ge(dma_oa, 32)

        @block.sync
        def _(sync):
            for j in range(NGROUPS):
                if j >= NB:
                    # all matmuls of group j-NB done -> ring slot is free
                    sync.wait_ge(mm_sem, 2 + MPG * (j - NB) + MPG)
                s0 = (j % NB) * GROUP_F
                sync.dma_start(
                    out=xb[:, s0 : s0 + GROUP_F], in_=x_d[j]
                ).then_inc(dma_x[j], 16)
            # final drain of pairs 0,1
            sync.wait_ge(cpv_sem, 2 * NGROUPS)
            sync.wait_ge(cps_sem, 2 * NGROUPS)
            for p in (0, 1):
                sync.dma_start(
                    out=out_d[2 * p : 2 * p + 2, :],
                    in_=out_sb[32 * p : 32 * p + 2, :],
                ).then_inc(dma_os, 16)
            sync.wait_ge(dma_os, 32)

        @block.vector
        def _(vector):
            vector.memset(dummy[:, :], 0.0).then_inc(ve_sem)
            vector.memset(ones_sb[:, :], 1.0).then_inc(ve_sem)
            vector.memset(s_sb[:, :], 0.0).then_inc(ve_sem)
            vector.wait_ge(dma_c, 16)
            # masked = (conn > 0.1) * conn
            vector.scalar_tensor_tensor(
                out=masked[:, :],
                in0=conn_sb[:, :],
                scalar=THRESHOLD,
                in1=conn_sb[:, :],
                op0=mybir.AluOpType.is_gt,
                op1=mybir.AluOpType.mult,
            ).then_inc(ve_sem)
            # S[0:64, 0] = 3 * w ; S[64:128, 1] = 3 * w  (block diagonal)
            vector.wait_ge(mm_sem, 2)
            vector.tensor_scalar_mul(s_sb[0:64, 0:1], psum[0:64, 0:1], STRENGTH
                                     ).then_inc(s_sem)
            vector.tensor_scalar_mul(s_sb[64:128, 1:2], psum[64:128, 0:1], STRENGTH
                                     ).then_inc(s_sem)
            for j in range(NGROUPS):
                for b2 in (0, 1):
                    vector.wait_ge(mm_sem, 2 + MPG * j + 4 * (b2 + 1))
                    src, dst = copy_aps(j, b2)
                    vector.tensor_copy(out=dst, in_=src).then_inc(cpv_sem)

        @block.tensor
        def _(tensor):
            # HAM warmup: ~3.5us of dummy matmuls so real work runs at 2.4 GHz.
            # M=98 also zero-fills psum[0:98] of every bank, which the wide
            # drain copies read (rows between the col-group pairs are junk).
            tensor.wait_ge(ve_sem, 1)
            for i in range(N_WARMUP):
                b = i % 8
                tensor.matmul(
                    psum[0:98, b * 512 : (b + 1) * 512],
                    dummy[:, 0:98],
                    dummy[:, :],
                    start=True,
                    stop=True,
                )
            tensor.wait_ge(ve_sem, 4)
            # w[j] = sum_i masked[i, j], materialized on partitions 0-63 and 64-127
            tensor.matmul(
                psum[0:64, 0:1], masked[:, :], ones_sb[:, :], start=True, stop=True
            ).then_inc(mm_sem)
            tensor.matmul(
                psum[64:128, 0:1],
                masked[:, :],
                ones_sb[:, :],
                start=True,
                stop=True,
                tile_position=(0, 64),
            ).then_inc(mm_sem)
            tensor.wait_ge(s_sem, 2)
            for j in range(NGROUPS):
                tensor.wait_ge(dma_x[j], 16)
                s0 = (j % NB) * GROUP_F
                for m in range(MPG):
                    if j >= NB:
                        # bank quad (4j..4j+3)%8 was drained by group j-NB's
                        # copies
                        if m == 0:
                            tensor.wait_ge(cpv_sem, 2 * (j - NB) + 2)
                        if m == 8:
                            tensor.wait_ge(cps_sem, 2 * (j - NB) + 2)
                    p = m % 4
                    b = (4 * j + m // 4) % 8
                    tensor.matmul(
                        psum[32 * p : 32 * p + 2, b * 512 : b * 512 + MM_F],
                        s_sb[:, :],
                        xb[:, s0 + m * MM_F : s0 + (m + 1) * MM_F],
                        start=True,
                        stop=True,
                        tile_position=(0, 32 * p),
                    ).then_inc(mm_sem)

    return nc


def shard_inputs(unit_outputs: np.ndarray, conn: np.ndarray):
    """Full inputs -> per-core in_maps with the group layout the kernel expects.

    Shards are pre-rounded to bf16 (what the device matmul consumes anyway)
    so the HBM stream moves half the bytes.
    """
    import ml_dtypes

    conn = np.ascontiguousarray(conn, dtype=np.float32)
    in_maps = []
    for c in range(NCORES):
        xc = np.ascontiguousarray(
            unit_outputs[:, :, c * NS : (c + 1) * NS], dtype=np.float32
        ).reshape(U, FLAT)
        # [u, j, h, f] -> [j, (h u), f]
        v = xc.reshape(U, NGROUPS, 2, GROUP_F)
        tiles = np.ascontiguousarray(
            v.transpose(1, 2, 0, 3).astype(ml_dtypes.bfloat16)
        ).reshape(NGROUPS, 128, GROUP_F)
        in_maps.append({"x": tiles, "conn": conn})
    return in_maps


def unshard_output(results) -> np.ndarray:
    """Per-core [8, 6400] bf16 outputs -> full [T, N] f32.

    Row 2p+h col j*1600 + b2*400 + cc holds matmul m = 4*b2+p of group j,
    i.e. flat = j*12800 + h*6400 + m*400 + cc.
    """
    final = np.empty((T, N), dtype=np.float32)
    for c in range(NCORES):
        r = np.asarray(results[c]["out"]).astype(np.float32)
        arr = r.reshape(4, 2, NGROUPS, 4, MM_F)  # [p, h, j, b2, cc]
        flat = arr.transpose(2, 1, 3, 0, 4).reshape(FLAT)  # [j, h, b2, p, cc]
        final[:, c * NS : (c + 1) * NS] = flat.reshape(T, NS)
    return final


_NC_CACHE = None


def kernel(unit_outputs: np.ndarray, conn: np.ndarray) -> np.ndarray:
    global _NC_CACHE
    if _NC_CACHE is None:
        _NC_CACHE = build_nc()
    in_maps = shard_inputs(unit_outputs, conn)
    res = run_bass_kernel_spmd(_NC_CACHE, in_maps, core_ids=list(range(NCORES)))
    return unshard_output(res.results)


if __name__ == "__main__":
    rng = np.random.default_rng(0)
    uo = rng.random((U, T, N), dtype=np.float32)
    cn = rng.random((U, U), dtype=np.float32)
    out = kernel(uo, cn)
    w = np.where(cn > THRESHOLD, cn, 0.0).sum(axis=0) * STRENGTH
    ref = np.einsum("j,jtn->tn", w, uo)
    err = np.abs(out - ref).max() / np.abs(ref).max()
    print("rel err:", err)


# revision 26
# speedup vs baseline: 1.3846x; 1.0206x over previous
"""Distributed Trainium2 kernel for gnn_message_passing (nn_AMN_18004502905276).

Reference computation:
    masked = where(conn > 0.1, conn, 0)          # [64, 64]
    w      = 3.0 * masked.sum(axis=0)            # [64]
    out    = einsum('j,jtn->tn', w, unit_outputs)  # [100, 4096]

Strategy: shard along N (4096 = 8 x 512) so every core computes its own
output slice with zero collectives.  Per core the weighted unit-sum is a
[128,2]^T @ [128,400] bf16 matmul: the moving operand stacks two 64-unit
groups on the 128 partitions, the stationary operand is a block-diagonal
copy of w (computed on device from conn).  Inputs are pre-rounded to bf16
host-side (the matmul consumes bf16 either way), halving the HBM stream.

Per core: 8 HWDGE DMAs of [128, 3200] bf16 feed 8 matmuls each.  Matmul m
rotates over PE column groups 32*(m%4) and banks (2j+m//4)%8, so four
consecutive matmuls write the SAME PSUM bank at partition pairs
0/32/64/96 — one [98, 400] DVE copy then drains all four results at once
(lanes in between move junk that is never DMAed).  VectorE copies the
first bank of each group, ScalarE the second; eight [2, *] HWDGE DMAs
(four early, four at the end) write the result rows to DRAM as bf16.
A burst of dummy matmuls at kernel start warms the PE (HAM K=8/8) and
zero-fills psum[0:98] of every bank for the wide copies.
"""

import contextlib
import sys

import numpy as np

sys.path.insert(0, "/opt/trn_rl_repo")

import concourse.bass as bass
import concourse.mybir as mybir
from concourse.bass_utils import run_bass_kernel_spmd

# Problem geometry (hardcoded per the harness contract).
U, T, N = 64, 100, 4096
NCORES = 8
NS = N // NCORES          # 512 output columns per core
FLAT = T * NS             # 51200 flat (t, n) positions per core
GROUP_F = 3200            # moving columns per DMA group half
NGROUPS = FLAT // (2 * GROUP_F)  # 8
MM_F = 400                # moving columns per matmul
MPG = 8                   # matmuls per group
NB = 4                    # group buffers in the SBUF ring
N_WARMUP = 8              # dummy matmuls: HAM warmup + PSUM bank init
EARLY = 6                 # groups whose output drains before the run ends
F32 = mybir.dt.float32
BF16 = mybir.dt.bfloat16

THRESHOLD = 0.1
STRENGTH = 3.0


def build_nc() -> bass.Bass:
    nc = bass.Bass()

    x_d = nc.declare_dram_parameter("x", [NGROUPS, 128, GROUP_F], BF16, isOutput=False)
    conn_d = nc.declare_dram_parameter("conn", [U, U], F32, isOutput=False)
    out_d = nc.declare_dram_parameter("out", [8, 6400], BF16, isOutput=True)

    ctx = contextlib.ExitStack()
    with ctx:
        xb = ctx.enter_context(nc.sbuf_tensor("xb", [128, NB * GROUP_F], BF16))
        dummy = ctx.enter_context(nc.sbuf_tensor([128, 512], BF16))
        conn_sb = ctx.enter_context(nc.sbuf_tensor([U, U], F32))
        masked = ctx.enter_context(nc.sbuf_tensor([U, U], F32))
        ones_sb = ctx.enter_context(nc.sbuf_tensor([U, 1], F32))
        s_sb = ctx.enter_context(nc.sbuf_tensor([128, 2], BF16))
        out_sb = ctx.enter_context(nc.sbuf_tensor([128, 6400], BF16))
        psum = ctx.enter_context(nc.psum_tensor([128, 4096], F32))

        ctx.enter_context(nc.Block())
        block = nc.cur_block
        dma_c = ctx.enter_context(nc.semaphore("dma_c"))
        dma_x = [
            ctx.enter_context(nc.semaphore(f"dma_x{i}")) for i in range(NGROUPS)
        ]
        dma_os = ctx.enter_context(nc.semaphore("dma_os"))
        dma_oa = ctx.enter_context(nc.semaphore("dma_oa"))
        mm_sem = ctx.enter_context(nc.semaphore("mm_sem"))
        ve_sem = ctx.enter_context(nc.semaphore("ve_sem"))
        s_sem = ctx.enter_context(nc.semaphore("s_sem"))
        cpv_sem = ctx.enter_context(nc.semaphore("cpv_sem"))
        cps_sem = ctx.enter_context(nc.semaphore("cps_sem"))

        EC = EARLY * 2 * MM_F  # columns covered by the early drain

        def copy_aps(j, half):
            """All four pairs' results for bank (2j+half)%8 of group j."""
            b = (2 * j + half) % 8
            src = psum[0:98, b * 512 : b * 512 + MM_F]
            c0 = j * 2 * MM_F + half * MM_F
            dst = out_sb[0:98, c0 : c0 + MM_F]
            return src, dst

        @block.scalar
        def _(scalar):
            # conn load on the ACT HWDGE ring so the SP ring starts on x
            scalar.dma_start(out=conn_sb[:, :], in_=conn_d[:, :]).then_inc(dma_c, 16)
            for j in range(NGROUPS):
                scalar.wait_ge(mm_sem, 2 + MPG * j + 8)
                src, dst = copy_aps(j, 1)
                scalar.copy(dst, src).then_inc(cps_sem)
            # drain pairs 2,3 (self-wait: the DMAs must not race scalar's
            # own in-flight copies)
            for lo, hi, need in ((0, EC, EARLY), (EC, 6400, NGROUPS)):
                scalar.wait_ge(cps_sem, need)
                scalar.wait_ge(cpv_sem, need)
                for p in (2, 3):
                    scalar.dma_start(
                        out=out_d[2 * p : 2 * p + 2, lo:hi],
                        in_=out_sb[32 * p : 32 * p + 2, lo:hi],
                    ).then_inc(dma_oa, 16)
            scalar.wait_ge(dma_oa, 64)

        @block.sync
        def _(sync):
            for j in range(NGROUPS):
                if j >= NB:
                    # all matmuls of group j-NB done -> ring slot is free
                    sync.wait_ge(mm_sem, 2 + MPG * (j - NB) + MPG)
                s0 = (j % NB) * GROUP_F
                sync.dma_start(
                    out=xb[:, s0 : s0 + GROUP_F], in_=x_d[j]
                ).then_inc(dma_x[j], 16)
            # drain pairs 0,1
            for lo, hi, need in ((0, EC, EARLY), (EC, 6400, NGROUPS)):
                sync.wait_ge(cpv_sem, need)
                sync.wait_ge(cps_sem, need)
                for p in (0, 1):
                    sync.dma_start(
                        out=out_d[2 * p : 2 * p + 2, lo:hi],
                        in_=out_sb[32 * p : 32 * p + 2, lo:hi],
                    ).then_inc(dma_os, 16)
            sync.wait_ge(dma_os, 64)

        @block.vector
        def _(vector):
            vector.memset(dummy[:, :], 0.0).then_inc(ve_sem)
            vector.memset(ones_sb[:, :], 1.0).then_inc(ve_sem)
            vector.memset(s_sb[:, :], 0.0).then_inc(ve_sem)
            vector.wait_ge(dma_c, 16)
            # masked = (conn > 0.1) * conn
            vector.scalar_tensor_tensor(
                out=masked[:, :],
                in0=conn_sb[:, :],
                scalar=THRESHOLD,
                in1=conn_sb[:, :],
                op0=mybir.AluOpType.is_gt,
                op1=mybir.AluOpType.mult,
            ).then_inc(ve_sem)
            # S[0:64, 0] = 3 * w ; S[64:128, 1] = 3 * w  (block diagonal)
            vector.wait_ge(mm_sem, 2)
            vector.tensor_scalar_mul(s_sb[0:64, 0:1], psum[0:64, 0:1], STRENGTH
                                     ).then_inc(s_sem)
            vector.tensor_scalar_mul(s_sb[64:128, 1:2], psum[64:128, 0:1], STRENGTH
                                     ).then_inc(s_sem)
            for j in range(NGROUPS):
                vector.wait_ge(mm_sem, 2 + MPG * j + 4)
                src, dst = copy_aps(j, 0)
                vector.tensor_copy(out=dst, in_=src).then_inc(cpv_sem)

        @block.tensor
        def _(tensor):
            # HAM warmup: ~3.5us of dummy matmuls so real work runs at 2.4 GHz.
            # M=98 also zero-fills psum[0:98] of every bank, which the wide
            # drain copies read (rows between the col-group pairs are junk).
            tensor.wait_ge(ve_sem, 1)
            for i in range(N_WARMUP):
                b = i % 8
                tensor.matmul(
                    psum[0:98, b * 512 : (b + 1) * 512],
                    dummy[:, 0:98],
                    dummy[:, :],
                    start=True,
                    stop=True,
                )
            tensor.wait_ge(ve_sem, 4)
            # w[j] = sum_i masked[i, j], materialized on partitions 0-63 and 64-127
            tensor.matmul(
                psum[0:64, 0:1], masked[:, :], ones_sb[:, :], start=True, stop=True
            ).then_inc(mm_sem)
            tensor.matmul(
                psum[64:128, 0:1],
                masked[:, :],
                ones_sb[:, :],
                start=True,
                stop=True,
                tile_position=(0, 64),
            ).then_inc(mm_sem)
            tensor.wait_ge(s_sem, 2)
            for j in range(NGROUPS):
                tensor.wait_ge(dma_x[j], 16)
                s0 = (j % NB) * GROUP_F
                for m in range(MPG):
                    if j >= NB:
                        # bank (2j+m//4)%8 was drained by group j-NB's copy
                        if m == 0:
                            tensor.wait_ge(cpv_sem, j - NB + 1)
                        if m == 4:
                            tensor.wait_ge(cps_sem, j - NB + 1)
                    p = m % 4
                    b = (2 * j + m // 4) % 8
                    tensor.matmul(
                        psum[32 * p : 32 * p + 2, b * 512 : b * 512 + MM_F],
                        s_sb[:, :],
                        xb[:, s0 + m * MM_F : s0 + (m + 1) * MM_F],
                        start=True,
                        stop=True,
                        tile_position=(0, 32 * p),
                    ).then_inc(mm_sem)

    return nc


def shard_inputs(unit_outputs: np.ndarray, conn: np.ndarray):
    """Full inputs -> per-core in_maps with the group layout the kernel expects.

    Shards are pre-rounded to bf16 (what the device matmul consumes anyway)
    so the HBM stream moves half the bytes.
    """
    import ml_dtypes

    conn = np.ascontiguousarray(conn, dtype=np.float32)
    in_maps = []
    for c in range(NCORES):
        xc = np.ascontiguousarray(
            unit_outputs[:, :, c * NS : (c + 1) * NS], dtype=np.float32
        ).reshape(U, FLAT)
        # [u, j, h, f] -> [j, (h u), f]
        v = xc.reshape(U, NGROUPS, 2, GROUP_F)
        tiles = np.ascontiguousarray(
            v.transpose(1, 2, 0, 3).astype(ml_dtypes.bfloat16)
        ).reshape(NGROUPS, 128, GROUP_F)
        in_maps.append({"x": tiles, "conn": conn})
    return in_maps


def unshard_output(results) -> np.ndarray:
    """Per-core [8, 6400] bf16 outputs -> full [T, N] f32.

    Row 2p+h col j*800 + b2*400 + cc holds matmul m = 4*b2+p of group j,
    i.e. flat = j*6400 + h*3200 + m*400 + cc.
    """
    final = np.empty((T, N), dtype=np.float32)
    for c in range(NCORES):
        r = np.asarray(results[c]["out"]).astype(np.float32)
        arr = r.reshape(4, 2, NGROUPS, 2, MM_F)  # [p, h, j, b2, cc]
        flat = arr.transpose(2, 1, 3, 0, 4).reshape(FLAT)  # [j, h, b2, p, cc]
        final[:, c * NS : (c + 1) * NS] = flat.reshape(T, NS)
    return final


_NC_CACHE = None


def kernel(unit_outputs: np.ndarray, conn: np.ndarray) -> np.ndarray:
    global _NC_CACHE
    if _NC_CACHE is None:
        _NC_CACHE = build_nc()
    in_maps = shard_inputs(unit_outputs, conn)
    res = run_bass_kernel_spmd(_NC_CACHE, in_maps, core_ids=list(range(NCORES)))
    return unshard_output(res.results)


if __name__ == "__main__":
    rng = np.random.default_rng(0)
    uo = rng.random((U, T, N), dtype=np.float32)
    cn = rng.random((U, U), dtype=np.float32)
    out = kernel(uo, cn)
    w = np.where(cn > THRESHOLD, cn, 0.0).sum(axis=0) * STRENGTH
    ref = np.einsum("j,jtn->tn", w, uo)
    err = np.abs(out - ref).max() / np.abs(ref).max()
    print("rel err:", err)
